# revision 3
# baseline (speedup 1.0000x reference)
"""GAT (2-layer, 8 heads) Trainium2 Bass kernel, sharded across 8 NeuronCores.

Sharding: nodes are partitioned into 8 contiguous ranges (graph parallel).
Edges are routed (on host) to the core that owns their dst node so that
segment-softmax and scatter-add stay local.  Each layer's dense part runs on
the owning core only; an AllGather then replicates the per-core
[h | al_src] rows (bf16) so the per-edge source-feature fetch is a local
dma_gather (int16 indices; the 50k-row table is split into two <32k-row
halves and each dst-tile's edge blocks are grouped by half).

v2 changes vs baseline:
  - per-edge al_dst is no longer dma_gathered; it is selected on the PE via
    a host-uploaded fp8 transposed selection matrix (S_T) against an SBUF
    copy of the dst tile's al_dst rows (written during the dense phase).
  - the per-edge scatter selection matrix S is also host-uploaded (fp8)
    instead of built on DVE with is_equal.
  - leaky-relu (and the flush relu) run on the Scalar engine (Lrelu/Relu).
  - the alpha-scaled rows are written to a fresh tile (hm) instead of
    scaling hs in place, so gathers only wait on the DVE stage.

Math note: the reference's segment-max subtraction is skipped — logits here
are O(1) so exp() cannot overflow, and alpha = e/z is invariant to the shift.

Self-contained: hardcodes all shapes from the problem spec.
"""

import os
import sys

import numpy as np

for _p in ("/opt/trn_rl_repo",):
    if _p not in sys.path and os.path.isdir(_p):
        sys.path.insert(0, _p)

import ml_dtypes

import concourse.bacc as bacc
import concourse.bass as bass
import concourse.tile as tile
from concourse import ap_utils, bass_utils, mybir
from concourse.masks import make_identity

# ---------------- problem constants (from spec) ----------------
N = 50000
D_IN = 256
HID = 32
HEADS = 8
D = HEADS * HID  # 256
NEG_SLOPE = 0.2
NCORES = 8

NSH = N // NCORES  # 6250 nodes per core
P = 128
NT = (NSH + P - 1) // P  # 49 dst tiles per core
NSH_PAD = NT * P  # 6272
NFULL = NCORES * NSH_PAD  # 50176 rows in the AllGathered table
# The table is split at the tile-25 boundary into two independently
# AllGathered halves (both < 2**15 rows, int16-indexable); the A half can be
# gathered as soon as tiles 0-24 of every core are done.
NTA = 25  # tiles in the A half
LOCA = NTA * P  # 3200 local rows in A
LOCB = NSH_PAD - LOCA  # 3072 local rows in B
HALFA = NCORES * LOCA  # 25600
HALFB = NCORES * LOCB  # 24576
ROW = D + HEADS  # 264 bf16 per edge ([h | al_src]) fed to the scatter
WCOL = D + 2 * HEADS  # 272 dense output cols ([h | al_src | al_dst])
G = int(os.environ.get("GSIZE", "32"))  # edge blocks (of 128 edges) per group
# FP8 stores the gathered table as [h fp8 | al_src bf16] (272 B rows at a
# 512 B stride) instead of all-bf16 (528 B rows at 768 B) — ~35% less gather
# and AllGather traffic. "1" = both layers (rel err ~2.4e-2, too high),
# "2" = layer-2 table only (h1 stays bf16; only the final aggregation sees
# fp8 noise).
_FP8MODE = os.environ.get("FP8", "2")
FP8L = {"0": (False, False), "1": (True, True),
        "2": (False, True)}[_FP8MODE]
# SDT8=0 uploads the selection matrices in bf16 instead of fp8 (debug/A-B)
SDT8 = os.environ.get("SDT8", "1") == "1"

f32 = mybir.dt.float32
bf16 = mybir.dt.bfloat16
f8 = mybir.dt.float8e4
i16 = mybir.dt.int16
bfnp = ml_dtypes.bfloat16
f8np = ml_dtypes.float8_e4m3

TDTl = [f8 if f else bf16 for f in FP8L]
ROWTl = [D + 2 * HEADS if f else ROW for f in FP8L]  # 272 fp8 / 264 bf16
ROWPTl = [512 if f else 384 for f in FP8L]  # row stride (256 B aligned)
SDT = f8 if SDT8 else bf16
SNP = f8np if SDT8 else bfnp


def _ant_dma_gather(gp, out_ap, in_ap, idxs_ap, num_idxs, elem_size, elem_step,
                    queue_num=0):
    """bass.dma_gather (non-transpose, HBM source) without the elem%256 assert.

    The row stride (elem_step elements) must still be a multiple of 256 B.
    out[p, j, :] = in[idxs[j*128 + p], :elem_size]
    """
    assert idxs_ap.dtype == mybir.dt.int16
    assert in_ap.dtype == out_ap.dtype
    assert idxs_ap.space == bass.MemorySpace.SBUF
    assert out_ap.space == bass.MemorySpace.SBUF
    assert in_ap.space == bass.MemorySpace.DRAM
    assert ap_utils.ap_is_contiguous(out_ap.ap[1:])
    assert ap_utils.ap_is_contiguous(idxs_ap.ap[1:])
    assert num_idxs % 128 == 0
    assert out_ap.ap[-1][1] == elem_size
    assert out_ap.ap[0][1] * out_ap.ap[1][1] == num_idxs
    assert in_ap.ap[0][0] == elem_step
    stride_bytes = elem_step * mybir.dt.size(in_ap.dtype)
    stride_bytes_256 = stride_bytes // 256
    assert stride_bytes_256 * 256 == stride_bytes and stride_bytes_256 < 256
    _in_ap = gp.lower_ap_dma(in_ap, for_custom_bir_dma=True)
    return gp.add_instruction(
        mybir.InstDMAGatherAnt(
            name=gp.bass.get_next_instruction_name(),
            ins=[*_in_ap, gp.lower_ap(idxs_ap),
                 gp.lower_val_access(gp.to_reg(num_idxs))],
            outs=[gp.lower_ap(out_ap)],
            transpose=False,
            num_idxs=num_idxs,
            elem_size=elem_size,
            stride_bytes_256=stride_bytes_256,
            gen_mode=0,
            single_packet=os.environ.get("SINGLE_PACKET", "1") == "1",
            queue_num=queue_num,
            sbuf_tokens_per_rank=0,
            sbuf_free_dim_per_rank=0,
            sbuf_free_dim_pad_per_rank=0,
            sbuf_byte_offset=0,
        )
    )


def _wrap16(vals_pb):
    """[128, btot] logical (partition, block) values -> [16, btot*8] int16
    dma_gather index layout (16-wrapped; replicate to 8 channel groups on
    device)."""
    btot = vals_pb.shape[1]
    flat = vals_pb.T.reshape(-1)  # logical position q = blk*128 + p
    return flat.reshape(btot * 8, 16).T.astype(np.int16)  # [16, btot*8]


# ---------------- host-side routing ----------------
def _route_edges(edge_index: np.ndarray):
    """Route edges (plus self-loops) to the core owning their dst; order each
    dst-tile's edges by src-table half so gather calls are half-homogeneous.

    Returns per-core index arrays plus the shared block/run schedule and the
    per-block selection matrices (S and S^T) as uint8.
    """
    src = np.concatenate([edge_index[0].astype(np.int64), np.arange(N, dtype=np.int64)])
    dst = np.concatenate([edge_index[1].astype(np.int64), np.arange(N, dtype=np.int64)])
    core = dst // NSH
    dloc = dst - core * NSH
    s_core, s_loc = src // NSH, src % NSH
    half = (s_loc >= LOCA).astype(np.int64)
    # row within the half's AllGathered table
    g_row = np.where(half == 0, s_core * LOCA + s_loc,
                     s_core * LOCB + (s_loc - LOCA))

    # per (core, tile, half) edge lists
    buckets = {}
    counts = np.zeros((NCORES, NT, 2), dtype=np.int64)
    for c in range(NCORES):
        m = core == c
        gr_c, dl_c, hf_c = g_row[m], dloc[m], half[m]
        key = dl_c // P * 2 + hf_c
        order = np.argsort(key, kind="stable")
        gr_c, dl_c, hf_c = gr_c[order], dl_c[order], hf_c[order]
        key = key[order]
        bounds = np.searchsorted(key, np.arange(NT * 2 + 1))
        for t in range(NT):
            for hf in range(2):
                lo, hi = bounds[t * 2 + hf], bounds[t * 2 + hf + 1]
                buckets[(c, t, hf)] = (gr_c[lo:hi], dl_c[lo:hi])
                counts[c, t, hf] = hi - lo

    B = -(-counts.max(axis=0) // P)  # [NT, 2] blocks per (tile, half)
    # block schedule: per tile, halves ordered to merge runs across tiles
    sched = []  # (tile, half)
    for t in range(NT):
        order = (0, 1) if t % 2 == 0 else (1, 0)
        for hf in order:
            sched.extend([(t, hf)] * int(B[t, hf]))
    # pad to a multiple of G, extending the final (tile, half) span
    btot = len(sched)
    pad = (-btot) % G
    sched.extend([sched[-1]] * pad)
    btot = len(sched)

    tile_of_block = np.array([t for t, _ in sched])
    half_of_block = np.array([hf for _, hf in sched])
    first = np.zeros(btot, dtype=bool)
    last = np.zeros(btot, dtype=bool)
    for t in range(NT):
        w = np.where(tile_of_block == t)[0]
        first[w[0]] = True
        last[w[-1]] = True

    # gather runs: maximal same-half spans, split at group boundaries
    runs = []  # (start_block, n_blocks, half)
    b = 0
    while b < btot:
        e = b + 1
        while (e < btot and half_of_block[e] == half_of_block[b]
               and e % G != 0):
            e += 1
        runs.append((b, e - b, int(half_of_block[b])))
        b = e

    # per-core per-(p, block) values
    srcv = np.zeros((NCORES, 128, btot), dtype=np.int64)
    dstl = np.full((NCORES, 128, btot), -1, dtype=np.int64)
    blk_start = {}
    for i, (t, hf) in enumerate(sched):
        if (t, hf) not in blk_start:
            blk_start[(t, hf)] = i
    for c in range(NCORES):
        for t in range(NT):
            for hf in range(2):
                gr, dl = buckets[(c, t, hf)]
                n_e = len(gr)
                if n_e == 0:
                    continue
                b0 = blk_start[(t, hf)]
                j = b0 * 128 + np.arange(n_e)
                srcv[c, j % 128, j // 128] = gr
                dstl[c, j % 128, j // 128] = dl - t * P
    # selection matrices: S[c, p, b, d] = (dstl[c, p, b] == d)
    smat = (dstl[:, :, :, None] == np.arange(P)[None, None, None, :])
    smat = smat.astype(np.uint8)                       # [C, 128e, btot, 128d]
    stmat = np.ascontiguousarray(smat.transpose(0, 3, 2, 1))  # [C, 128d, btot, 128e]
    return srcv, smat, stmat, tile_of_block, first, last, runs, btot


# ---------------- device program ----------------
def _build_program(tile_of_block, first, last, runs, btot, phases=5):
    import contextlib

    nc = bacc.Bacc(
        "TRN2",
        target_bir_lowering=False,
        debug=False,
        enable_asserts=False,
        num_devices=NCORES,
        num_swdge_queues=4,
    )
    ngroups = btot // G
    qrr = [0]  # round-robin SWDGE queue assignment for gather calls

    def next_q(kind=0):
        qrr[0] = (qrr[0] + 1) % 4
        return qrr[0]

    # I/O (xT, weights, and tables in bf16; biases f32; S matrices fp8)
    xT = nc.dram_tensor("xT", [D_IN, NSH_PAD], bf16, kind="ExternalInput")
    w1 = nc.dram_tensor("w1", [D_IN, WCOL], bf16, kind="ExternalInput")
    w2 = nc.dram_tensor("w2", [D, WCOL], bf16, kind="ExternalInput")
    b1d = nc.dram_tensor("b1", [P, D], f32, kind="ExternalInput")
    b2d = nc.dram_tensor("b2", [P, HID], f32, kind="ExternalInput")
    srcd = nc.dram_tensor("srci", [16, btot * 8], i16, kind="ExternalInput")
    sd = nc.dram_tensor("smat", [P, btot * P], SDT, kind="ExternalInput")
    std = nc.dram_tensor("stmat", [P, btot * P], SDT, kind="ExternalInput")
    outd = nc.dram_tensor("out", [NSH_PAD, HID], f32, kind="ExternalOutput")

    # internal DRAM (A/B halves are separate tensors so each AllGather's
    # dependency covers only the dense tiles that feed it)
    haug_sh = [
        [nc.dram_tensor(f"haug{l}_sh{h}", [loc, ROWPTl[l - 1]], TDTl[l - 1],
                        kind="Internal")
         for h, loc in ((0, LOCA), (1, LOCB))]
        for l in (1, 2)
    ]
    haug_full = [
        [nc.dram_tensor(
            f"haug{l}_full{h}", [rows, ROWPTl[l - 1]], TDTl[l - 1],
            kind="Internal", addr_space="Shared",
        ) for h, rows in ((0, HALFA), (1, HALFB))]
        for l in (1, 2)
    ]

    rg = [list(range(NCORES))]

    with tile.TileContext(nc) as tc, contextlib.ExitStack() as ctx:
        const = ctx.enter_context(tc.tile_pool(name="const", bufs=1))
        sb = ctx.enter_context(tc.tile_pool(name="sb", bufs=2))
        sb3 = ctx.enter_context(tc.tile_pool(name="sb3", bufs=3))
        sb4 = ctx.enter_context(tc.tile_pool(name="sb4", bufs=4))
        ps = ctx.enter_context(tc.tile_pool(name="ps", bufs=2, space="PSUM"))

        # constants
        ident = const.tile([P, P], f32)
        make_identity(nc, ident[:])
        w1t = const.tile([P, 2, WCOL], bf16)
        nc.sync.dma_start(w1t[:], w1.ap().rearrange("(k p) n -> p k n", p=P))
        w2t = const.tile([P, 2, WCOL], bf16)
        nc.sync.dma_start(w2t[:], w2.ap().rearrange("(k p) n -> p k n", p=P))
        b1t = const.tile([P, D], f32)
        nc.sync.dma_start(b1t[:], b1d.ap())
        b2t = const.tile([P, HID], f32)
        nc.sync.dma_start(b2t[:], b2d.ap())
        # leaky-relu slope as a per-partition constant (tensor_scalar with an
        # immediate measured ~2.9us/call on DVE; broadcast TT is ~0.36us)
        nst = const.tile([P, 1], f32)
        nc.vector.memset(nst[:], NEG_SLOPE)
        # z-clamp: the last tile's padding partitions (nodes >= NSH) receive
        # no edges, so z=0 there; 1/z=inf would turn into 0*inf=nan in h1 and
        # poison the layer-2 ad matmul (PE reduces over all 128 partitions
        # and 0*nan=nan). Clamping z keeps those rows finite.
        tnt = const.tile([P, 1], f32)
        nc.vector.memset(tnt[:], 1e-30)
        srci_sb = const.tile([128, btot * 8], i16)
        for r in range(8):
            nc.sync.dma_start(srci_sb[r * 16:(r + 1) * 16, :], srcd.ap())
        # al_dst rows for the core's own dst tiles, per layer: [128, t, h]
        # (separate tiles so edge1's reads don't alias dense2's writes)
        adst_sb = [const.tile([P, NT, HEADS], bf16, name=f"adst{l}")
                   for l in (0, 1)]

        def dense_tile(nt, lhsT_k0, lhsT_k1, wt, layer):
            """[128 nodes] x Waug matmul -> write haug shard rows + adst_sb."""
            d_ps = ps.tile([P, WCOL], f32, tag="dmm", name="d_ps")
            nc.tensor.matmul(d_ps[:], lhsT=lhsT_k0, rhs=wt[:, 0, :], start=True,
                             stop=False)
            nc.tensor.matmul(d_ps[:], lhsT=lhsT_k1, rhs=wt[:, 1, :], start=False,
                             stop=True)
            hb = sb.tile([P, ROWTl[layer]], TDTl[layer], tag="hb", name="hb")
            if FP8L[layer]:
                nc.vector.tensor_copy(hb[:, 0:D], d_ps[:, 0:D])
                nc.vector.tensor_copy(hb[:, D:].bitcast(bf16),
                                      d_ps[:, D:ROW])
            else:
                nc.vector.tensor_copy(hb[:], d_ps[:, 0:ROW])
            nc.vector.tensor_copy(adst_sb[layer][:, nt, :], d_ps[:, ROW:WCOL])
            hf, base = (0, nt * P) if nt < NTA else (1, (nt - NTA) * P)
            nc.sync.dma_start(
                haug_sh[layer][hf].ap()[base:base + P, 0:ROWTl[layer]], hb[:])

        def all_gather(layer, hf):
            nc.gpsimd.collective_compute(
                "AllGather",
                mybir.AluOpType.bypass,
                replica_groups=rg,
                ins=[haug_sh[layer][hf].ap()],
                outs=[haug_full[layer][hf].ap()],
            )

        # ---- dense layer 1: own shard only ([h | al_src | al_dst]);
        # the A-half AllGather is issued as soon as tiles 0..NTA-1 are done
        with nc.named_scope("dense1"):
            xTr = xT.ap().rearrange("(k p) n -> p k n", p=P)
            for nt in range(NT):
                xt = sb.tile([P, 2, P], bf16, tag="xt", name="xt")
                nc.sync.dma_start(xt[:], xTr[:, :, nt * P:(nt + 1) * P])
                dense_tile(nt, xt[:, 0, :], xt[:, 1, :], w1t, 0)
                if nt == NTA - 1 and phases >= 2:
                    all_gather(0, 0)

        # SWDGE descriptor ring holds 128 descs/engine shared by in-flight
        # calls; 8 blocks = 64/engine so two calls fit. 12+ hangs the ring.
        CHUNK = int(os.environ.get("CHUNK", "8"))  # blocks per dma_gather call

        # group after which all A-half dst tiles (0..NTA-1) have been flushed
        agA_group = max(
            b for b in range(btot) if last[b] and tile_of_block[b] == NTA - 1
        ) // G

        def edge_phase(layer, flush, mid_hook=None):
            acc = {}
            run_i = 0
            for q in range(ngroups):
                qs = q * G
                hs = sb4.tile([128, G, ROWTl[layer]], TDTl[layer], tag="hs",
                              name="hs")
                while run_i < len(runs) and runs[run_i][0] < qs + G:
                    b0, nb, hf = runs[run_i]
                    for c0 in range(0, nb, CHUNK):
                        cb0, cnb = b0 + c0, min(CHUNK, nb - c0)
                        _ant_dma_gather(
                            nc.gpsimd,
                            out_ap=hs[:, cb0 - qs:cb0 - qs + cnb, :],
                            in_ap=haug_full[layer][hf].ap(),
                            idxs_ap=srci_sb[:, cb0 * 8:(cb0 + cnb) * 8],
                            num_idxs=cnb * 128,
                            elem_size=ROWTl[layer],
                            elem_step=ROWPTl[layer],
                            queue_num=next_q(),
                        )
                    run_i += 1
                # selection matrices for this group (host-built, fp8)
                s_sb = sb3.tile([128, G, P], SDT, tag="s", name="s")
                nc.sync.dma_start(s_sb[:], sd.ap()[:, qs * P:(qs + G) * P])
                st_sb = sb3.tile([128, G, P], SDT, tag="st", name="st")
                nc.sync.dma_start(st_sb[:], std.ap()[:, qs * P:(qs + G) * P])
                # per-edge al_dst via PE select: ad[e, h] = sum_d S_T[d, e] adst[d, h]
                ad_ps = ps.tile([128, G, HEADS], f32, tag="adps", name="ad_ps")
                for g in range(G):
                    t = int(tile_of_block[qs + g])
                    nc.tensor.matmul(
                        ad_ps[:, g, :], lhsT=st_sb[:, g, :],
                        rhs=adst_sb[layer][:, t, :], start=True, stop=True,
                    )
                # logits = leaky_relu(al_src[src] + al_dst[dst]); w = exp(logits)
                asrc = (hs[:, :, D:].bitcast(bf16) if FP8L[layer]
                        else hs[:, :, D:ROW])
                lg = sb3.tile([128, G, HEADS], f32, tag="lg", name="lg")
                nc.vector.tensor_tensor(
                    out=lg[:], in0=asrc, in1=ad_ps[:],
                    op=mybir.AluOpType.add,
                )
                lr = sb3.tile([128, G, HEADS], f32, tag="lr", name="lr")
                nc.vector.tensor_tensor(
                    out=lr[:], in0=lg[:],
                    in1=nst[:].unsqueeze(2).to_broadcast([128, G, HEADS]),
                    op=mybir.AluOpType.mult,
                )
                nc.vector.tensor_tensor(
                    out=lr[:], in0=lg[:], in1=lr[:], op=mybir.AluOpType.max
                )
                # hm = [h * w | w]; exp writes w straight into hm's tail
                hm = sb3.tile([128, G, ROW], bf16, tag="hm", name="hm")
                nc.scalar.activation(hm[:, :, D:ROW], lr[:],
                                     mybir.ActivationFunctionType.Exp)
                nc.vector.tensor_tensor(
                    out=hm[:, :, 0:D].rearrange("p g (h c) -> p g h c", c=HID),
                    in0=hs[:, :, 0:D].rearrange("p g (h c) -> p g h c", c=HID),
                    in1=hm[:, :, D:ROW].unsqueeze(3).to_broadcast(
                        [128, G, HEADS, HID]),
                    op=mybir.AluOpType.mult,
                )
                for g in range(G):
                    b = qs + g
                    t = int(tile_of_block[b])
                    if first[b]:
                        acc[t] = ps.tile([P, ROW], f32, tag="acc", name=f"acc{t}")
                    nc.tensor.matmul(
                        acc[t][:], lhsT=s_sb[:, g, :], rhs=hm[:, g, :],
                        start=bool(first[b]), stop=bool(last[b]),
                    )
                    if last[b]:
                        flush(t, acc.pop(t))
                if mid_hook is not None and q == agA_group:
                    mid_hook()

        # ---- layer-1 flush: normalize + relu, then fused dense layer 2 ----
        def flush1(t, acc_ps):
            rz = sb.tile([P, HEADS], f32, tag="rz", name="rz")
            nc.vector.tensor_tensor(
                out=rz[:], in0=acc_ps[:, D:ROW],
                in1=tnt[:].to_broadcast([P, HEADS]),
                op=mybir.AluOpType.max,
            )
            nc.vector.reciprocal(rz[:], rz[:])
            h1 = sb3.tile([P, D], f32, tag="h1", name="h1")
            nc.vector.tensor_tensor(
                out=h1[:].rearrange("p (h c) -> p h c", c=HID),
                in0=acc_ps[:, 0:D].rearrange("p (h c) -> p h c", c=HID),
                in1=rz[:].unsqueeze(2).to_broadcast([P, HEADS, HID]),
                op=mybir.AluOpType.mult,
            )
            nc.vector.tensor_tensor(
                out=h1[:], in0=h1[:], in1=b1t[:], op=mybir.AluOpType.add
            )
            nc.scalar.activation(h1[:], h1[:],
                                 mybir.ActivationFunctionType.Relu)
            # transpose h1 -> lhsT tiles for the layer-2 dense matmul
            tp0 = ps.tile([P, P], f32, tag="tp", name="tp0")
            nc.tensor.transpose(tp0[:], h1[:, 0:P], ident[:])
            l0 = sb.tile([P, P], bf16, tag="l0", name="l0")
            nc.vector.tensor_copy(l0[:], tp0[:])
            tp1 = ps.tile([P, P], f32, tag="tp", name="tp1")
            nc.tensor.transpose(tp1[:], h1[:, P:D], ident[:])
            l1 = sb.tile([P, P], bf16, tag="l1", name="l1")
            nc.vector.tensor_copy(l1[:], tp1[:])
            dense_tile(t, l0[:], l1[:], w2t, 1)

        # ---- layer-2 flush: normalize, mean over heads, + b2, store ----
        # NOTE: the 1/HEADS mean factor is folded into w2's h-columns on the
        # host, so flush2 only divides by z and sums heads.
        def flush2(t, acc_ps):
            rz = sb.tile([P, HEADS], f32, tag="rz", name="rz")
            nc.vector.reciprocal(rz[:], acc_ps[:, D:ROW])
            t2 = sb.tile([P, D], f32, tag="t2", name="t2")
            nc.vector.tensor_tensor(
                out=t2[:].rearrange("p (h c) -> p h c", c=HID),
                in0=acc_ps[:, 0:D].rearrange("p (h c) -> p h c", c=HID),
                in1=rz[:].unsqueeze(2).to_broadcast([P, HEADS, HID]),
                op=mybir.AluOpType.mult,
            )
            o = sb.tile([P, HID], f32, tag="o", name="o")
            nc.vector.tensor_reduce(
                out=o[:],
                in_=t2[:].rearrange("p (h c) -> p c h", c=HID),
                axis=mybir.AxisListType.X,
                op=mybir.AluOpType.add,
            )
            nc.vector.tensor_tensor(
                out=o[:], in0=o[:], in1=b2t[:], op=mybir.AluOpType.add
            )
            nc.sync.dma_start(outd.ap()[t * P:(t + 1) * P, :], o[:])

        if phases >= 2:
            with nc.named_scope("ag1"):
                all_gather(0, 1)
        if phases >= 3:
            with nc.named_scope("edge1"):
                # ag2-A is issued mid-phase once tiles 0..NTA-1 are flushed
                edge_phase(0, flush1,
                           mid_hook=(lambda: all_gather(1, 0))
                           if phases >= 4 else None)
        if phases >= 4:
            with nc.named_scope("ag2"):
                all_gather(1, 1)
        if phases >= 5:
            with nc.named_scope("edge2"):
                edge_phase(1, flush2)

    nc.compile()
    return nc


# ---------------- public entry point ----------------
_CACHE = {}


def _prepare(edge_index):
    key = edge_index.tobytes()[:1024], int(edge_index.sum())
    if _CACHE.get("key") == key:
        return _CACHE["val"]
    srcv, smat, stmat, tile_of_block, first, last, runs, btot = _route_edges(
        np.asarray(edge_index)
    )
    nc = _build_program(tile_of_block, first, last, runs, btot)
    _CACHE["key"] = key
    _CACHE["val"] = (srcv, smat, stmat, btot, nc)
    return _CACHE["val"]


def _waug(W, a_src, a_dst):
    W = np.asarray(W, np.float32)
    asrc_m = np.zeros((D, HEADS), np.float32)
    adst_m = np.zeros((D, HEADS), np.float32)
    for h in range(HEADS):
        asrc_m[h * HID:(h + 1) * HID, h] = np.asarray(a_src, np.float32)[h]
        adst_m[h * HID:(h + 1) * HID, h] = np.asarray(a_dst, np.float32)[h]
    return np.concatenate([W, W @ asrc_m, W @ adst_m], axis=1)


def _make_in_maps(inputs, srcv, smat, stmat):
    x = np.asarray(inputs["x"], np.float32)
    w1_np = _waug(inputs["W1"], inputs["a_src1"], inputs["a_dst1"]).astype(bfnp)
    w2_np = _waug(inputs["W2"], inputs["a_src2"], inputs["a_dst2"])
    # fold the head-mean 1/HEADS into the h columns (z logits untouched)
    w2_np[:, 0:D] *= 1.0 / HEADS
    w2_np = w2_np.astype(bfnp)
    b1_np = np.tile(np.asarray(inputs["b1"], np.float32).reshape(1, D), (P, 1))
    b2_np = np.tile(np.asarray(inputs["b2"], np.float32).reshape(1, HID), (P, 1))

    xt_full = np.ascontiguousarray(x.T)  # [256, 50000]
    xT = np.zeros((NCORES, D_IN, NSH_PAD), bfnp)
    for c in range(NCORES):
        xT[c, :, :NSH] = xt_full[:, c * NSH:(c + 1) * NSH].astype(bfnp)

    btot = srcv.shape[2]
    return [
        {
            "xT": np.ascontiguousarray(xT[c]),
            "w1": w1_np,
            "w2": w2_np,
            "b1": b1_np,
            "b2": b2_np,
            "srci": _wrap16(srcv[c]),
            "smat": np.ascontiguousarray(
                smat[c].reshape(128, btot * P).astype(SNP)
            ),
            "stmat": np.ascontiguousarray(
                stmat[c].reshape(128, btot * P).astype(SNP)
            ),
        }
        for c in range(NCORES)
    ]


def kernel(
    x, edge_index, W1, a_src1, a_dst1, b1, W2, a_src2, a_dst2, b2
) -> np.ndarray:
    inputs = dict(x=x, W1=W1, a_src1=a_src1, a_dst1=a_dst1, b1=b1,
                  W2=W2, a_src2=a_src2, a_dst2=a_dst2, b2=b2)
    srcv, smat, stmat, btot, nc = _prepare(np.asarray(edge_index))
    in_maps = _make_in_maps(inputs, srcv, smat, stmat)
    res = bass_utils.run_bass_kernel_spmd(nc, in_maps, core_ids=list(range(NCORES)))
    out = np.concatenate(
        [np.asarray(res.results[c]["out"])[:NSH] for c in range(NCORES)], axis=0
    )
    return out
